# revision 37
# baseline (speedup 1.0000x reference)
"""Trainium2 Bass kernel for batched multi-head attention with per-head
clamped-exp temperature (nn_Attention_91173565760008).

  reference:
    scale = exp(min(logit_scale, ln(100)))          # [H,1,1]
    dots  = einsum('bhnd,bhmd->bhnm', q, k) * scale
    attn  = softmax(dots, -1)
    out   = einsum('bhnm,bhmd->bhnd', attn, v)

Shapes: B=4, H=12, N=2048, D=64, fp32.  8 NeuronCores, (B*H)=48 head-pairs
sharded 6 per core (data + head parallel, per the sharding hint).

Per-core device pipeline, per (b,h) pair. The key (m) axis is processed in
two 1024-wide halves with a flash-attention-style merge done on the host:

  for each query tile (128 rows) and each m-half:
    S_h  = (scale*Q)^T K_h       PE fp32r matmuls, natural [nq, m] layout
    nm_h = -rowmax(S_h)          DVE tensor_reduce(max, negate)
    P_h  = exp(S_h + nm_h)       ScalarE activation -> fp16
  per (group of 4 q-tiles, m-half): ONE 1MB xbar-transpose DMA of its P tiles
  (HW-benched: per-DMA fixed cost made the transpose ring the critical chain
  at finer granularity; a single 2MB transpose per group corrupts on HW)
  per group and m-half:
    O_h^T += V'_h^T P_h^T        PE fp16 matmuls, fp32 PSUM accumulation;
                                 V' has a ones column so O^T row 64 is the
                                 softmax denominator (rowsum) for free.
                                 Outputs pack 4-per-PSUM-bank (o_slots) and
                                 drain via one batched ACT copy -> fp16 SBUF
  outputs: O^T (66 rows: numerator + rowsum + pad, unnormalized), negmax
  host: out = (g0*O_0 + g1*O_1) / (g0*rs_0 + g1*rs_1),  g_h = exp(m_h - m)

The emission is software-pipelined `depth` groups deep (S/softmax phase of
unit u before the PV phase of unit u-depth) so the in-order PE stream never
parks behind the exp->transpose->PV chain.

DMA queue discipline (hardware requirement, see memory notes): the sync (SP)
HWDGE ring carries ONLY xbar transposes -- mixing plain DMAs into that ring
deterministically corrupts PV accumulations (exact-2x doubling). Loads and
stores ride the scalar (ACT) HWDGE ring.

Host-side work is layout marshalling (transposes/tiling), the 12-element
scale computation, and the final two-way merge + divide.
"""

import sys

sys.path.insert(0, "/opt/trn_rl_repo")

import numpy as np

B, H, N, D = 4, 12, 2048, 64
NCORES = 8
PAIRS = (B * H) // NCORES  # 6 head-pairs per core
QTILES = N // 128  # 16 query tiles of 128 rows per pair
GROUP = 2  # q-tiles per PV group (nq=256 per PV matmul)
NH = 2  # m halves
MH = N // NH  # 1024 columns per half
MT = MH // 128  # 8 m-tiles per half
DR = D + 2  # PV rows: 64 numerator + rowsum (ones col of V') + zero pad (even M)
MAX_LOG_SCALE = 4.6052  # ln(100) clamp from the module

_CACHE = {}

# pipeline-structure knobs (HW-benched 2026-08-09; 328069 ns vs 417644 baseline)
# group=4 + btile=4: PV matmuls 4x wider (N=512, LDWEIGHTS amortized on HW) and
# the P->P^T xbar transposes batch 4 q-tiles into ONE 1MB DMA per (group, half)
# -- the per-DMA ~0.6-0.7us HWDGE fixed cost was ~45% of the sync-ring time at
# [128,1024] granularity, and ablations showed the transpose ring was the #1
# critical chain (~146us exposed). 1MB transposes are HW-correct; a single 2MB
# transpose per group (tsplit=False path) CORRUPTS on HW -- do not enlarge.
# PSUM budget at group=4: s tiles [128,1024]f32 = 2 banks x s_bufs(3) + o tiles
# [66,512]f32 = 1 bank x o_bufs(2) = 8 banks exactly.
CFG = dict(
    group=4,  # q-tiles per PV group
    tsplit=True,  # (unused when btile>0) True: one transpose per (q-tile,half)
    p_bufs=3,
    pt_bufs=5,  # depth+2: one unit of slack beyond the PV that frees each
    # buffer, so transposes never stall the sync ring on PV completion
    # (pt_bufs=4=depth+1 measured 339us, =5 measured 328us; p_bufs=4 regressed)
    s_bufs=3,
    o_bufs=2,
    depth=3,  # groups between transpose issue and PV consumption
    ceng="scalar",  # engine for PSUM->SBUF output copies (GPSIMD cannot touch PSUM)
    pf_g=3,  # next-pair prefetch point; MUST be >= depth (future-reader safety)
    o_slots=1,  # PV outputs packed per PSUM bank before one batched copy
    fold=0,  # of every 24 max-reduces, this many use the GpSimd prefold
    ptg2=False,  # grp>1: [128,NH,MT,grp,128] P^T layout -> contiguous PV rhs
    btile=4,  # >0: h-major p/pt layout, transpose b q-tiles per DMA (1,2,4)
    ldeng="scalar",  # queue for plain loads/stores: "scalar" (HWDGE) | "gpsimd"
    io_bufs=2,  # qk/v input double-buffering depth
    ob_bufs=2,  # ot/nm output double-buffering depth
    noreduce=False,  # ABLATION ONLY: skip max-reduce, exp bias=-300 (wrong!)
    notranspose=False,  # ABLATION ONLY: skip P^T transposes (PV reads stale)
    pvfirst=False,  # emit pv_phase(u-depth) BEFORE s_phase(u): the o-drain
    # then only waits on PV matmuls (not QK+PV) in the in-order PE queue,
    # reducing head-of-line blocking of the next exps behind the ACT drain
    drain_lag=0,  # delay o-drain emission by this many units (needs o_bufs
    # >= 2*(1+lag) PSUM banks; pay with s_bufs=2 at lag=1)
    tring=False,  # alternate transpose DMAs between sync and scalar HWDGE
    # rings (halves per-ring FIFO serialization; loads share the scalar ring)
    hout=False,  # btile: h-outer (j-inner) S loop; issue each half's
    # transpose right after its 4 exps instead of after all 8
)

# test-time A/B override: BASS_CFG='{"group": 1, "depth": 6, ...}'
import json as _json
import os as _os

if _os.environ.get("BASS_CFG"):
    CFG.update(_json.loads(_os.environ["BASS_CFG"]))


def _build_nc(reps=1, **over):
    """Build + compile the single-core Tile program (same program runs SPMD
    on all 8 cores with different data).  reps>1 repeats the whole
    computation (same I/O) -- used only to measure marginal kernel time."""
    from contextlib import ExitStack

    import concourse.bacc as bacc
    import concourse.tile as tile
    from concourse import mybir

    f32 = mybir.dt.float32
    f32r = mybir.dt.float32r
    f16 = mybir.dt.float16

    nc = bacc.Bacc("TRN2", target_bir_lowering=False, debug=False)

    qk_d = nc.dram_tensor("qk", [PAIRS, D, 2, N], f32r, kind="ExternalInput")
    vt_d = nc.dram_tensor("vt", [PAIRS, 128, QTILES, DR], f16, kind="ExternalInput")
    ot_d = nc.dram_tensor("ot", [PAIRS, DR, NH, N], f16, kind="ExternalOutput")
    nm_d = nc.dram_tensor("nm", [PAIRS, 128, QTILES, NH], f32, kind="ExternalOutput")
    pt_dbg = None
    if over.get("dbg"):
        pt_dbg = nc.dram_tensor(
            "ptdbg", [QTILES, 128, NH, MT, 128], f16, kind="ExternalOutput"
        )

    cfg = dict(CFG)
    cfg.update(over)
    grp = cfg["group"]
    ngrp = QTILES // grp

    with ExitStack() as ctx:
        tc = ctx.enter_context(tile.TileContext(nc))
        io_pool = ctx.enter_context(tc.tile_pool(name="io", bufs=cfg["io_bufs"]))
        p_pool = ctx.enter_context(tc.tile_pool(name="p", bufs=cfg["p_bufs"]))
        pt_pool = ctx.enter_context(tc.tile_pool(name="pt", bufs=cfg["pt_bufs"]))
        ob_pool = ctx.enter_context(tc.tile_pool(name="ob", bufs=cfg["ob_bufs"]))
        fm_pool = ctx.enter_context(tc.tile_pool(name="fm", bufs=3))
        ps_s = ctx.enter_context(
            tc.tile_pool(name="ps_s", bufs=cfg["s_bufs"], space="PSUM")
        )
        ps_o = ctx.enter_context(
            tc.tile_pool(name="ps_o", bufs=cfg["o_bufs"], space="PSUM")
        )

        seq = [(r, p) for r in range(reps) for p in range(PAIRS)]

        ldq = nc.gpsimd if cfg["ldeng"] == "gpsimd" else nc.scalar

        def load_pair(idx):
            r, p = seq[idx]
            qk_sb = io_pool.tile([D, 2, N], f32r, tag="qk", name=f"qk_sb{r}_{p}")
            ldq.dma_start(qk_sb[:], qk_d[p])
            v_sb = io_pool.tile([128, QTILES, DR], f16, tag="v", name=f"v_sb{r}_{p}")
            ldq.dma_start(v_sb[:], vt_d[p])
            return qk_sb, v_sb

        # one work unit = (rep, pair, group). The S/softmax phase of unit u
        # is emitted BEFORE the PV phase of unit u-1 so the in-order PE
        # stream never parks behind the exp->transpose->PV chain of the
        # previous group.
        units = [
            (r, p, g) for r in range(reps) for p in range(PAIRS) for g in range(ngrp)
        ]
        pair_sb = {}  # (r,p) -> (qk_sb, v_sb, ot_sb, nm_sb)
        pend = {}  # unit idx -> (ptg, v_sb, ot_sb, nm_sb)

        pair_sb[seq[0]] = load_pair(0)
        nonlocal_ctr = [0]
        cbias = None
        if cfg["noreduce"]:
            cb_pool = ctx.enter_context(tc.tile_pool(name="cb", bufs=1))
            cbias = cb_pool.tile([128, 1], mybir.dt.float32, tag="cb")
            nc.vector.memset(cbias[:], -300.0)

        def s_phase(ui):
            r, p, g = units[ui]
            pidx = r * PAIRS + p
            if g == 0 and (r, p) not in pair_sb:
                pass  # loaded via prefetch below
            qk_sb, v_sb = pair_sb[(r, p)][:2]
            if g == 0:
                ot_sb = ob_pool.tile([DR, NH, N], f16, tag="ot")
                nm_sb = ob_pool.tile([128, QTILES, NH], f32, tag="nm")
                if cfg["noreduce"]:
                    nc.vector.memset(nm_sb[:], 0.0)  # keep store path valid
                pair_sb[(r, p)] = (qk_sb, v_sb, ot_sb, nm_sb)
            else:
                ot_sb, nm_sb = pair_sb[(r, p)][2:]
            if g == cfg["pf_g"] and pidx + 1 < len(seq):
                # prefetch next pair's inputs mid-pair -- late enough that
                # every reader of the SBUF buffer being recycled has already
                # been EMITTED (the tile framework cannot depend on future
                # readers: with pipeline depth D the previous pair's last PV
                # phases are emitted D units into this pair)
                pair_sb[seq[pidx + 1]] = load_pair(pidx + 1)

            bt = cfg["btile"]
            if bt:
                # h-major: transpose covers b contiguous q-tiles in ONE DMA
                p_grp = p_pool.tile([128, NH, grp, MH], f16, tag="p")
                ptg = pt_pool.tile([128, NH, grp, MT, 128], f16, tag="pt")
            else:
                p_grp = p_pool.tile([128, grp, NH, MH], f16, tag="p")
                if cfg["ptg2"]:
                    # [q2, h, t, j, i2]: PV rhs ptg[:, h, t] is contiguous
                    ptg = pt_pool.tile([128, NH, MT, grp, 128], f16, tag="pt")
                else:
                    ptg = pt_pool.tile([128, grp, NH, MT, 128], f16, tag="pt")
            jh_list = (
                [(jx, hx) for hx in range(NH) for jx in range(grp)]
                if (bt and cfg["hout"])
                else [(jx, hx) for jx in range(grp) for hx in range(NH)]
            )
            for j, h in jh_list:
                i = grp * g + j  # q-tile index
                if True:
                    # S[nq=128, m=1024] = (scale Q)^T K_h, fp32r on PE
                    s = ps_s.tile([128, MH], f32, tag="s")
                    for b in range(MH // 512):
                        mo = MH * h + 512 * b
                        nc.tensor.matmul(
                            s[:, 512 * b : 512 * (b + 1)],
                            lhsT=qk_sb[:, 0, 128 * i : 128 * (i + 1)],
                            rhs=qk_sb[:, 1, mo : mo + 512],
                            start=True,
                            stop=True,
                        )
                    # negmax_h = -max_m(S_h) per query row (kept for host).
                    # ISA allows only one PSUM operand per instruction, so
                    # the halved-scan trick runs as a GpSimd two-step prefold
                    # (copy + elementwise max, each with one PSUM input) on a
                    # tunable fraction of tiles; DVE reduces the folded half.
                    nmv = nm_sb[:, i, h : h + 1]
                    nonlocal_ctr[0] += 1
                    if cfg["noreduce"]:
                        pass  # ablation: no reduce; bias const below
                    elif nonlocal_ctr[0] % 24 < cfg["fold"]:
                        fold = fm_pool.tile([128, MH // 2], f32, tag="fm")
                        nc.gpsimd.tensor_copy(fold[:], s[:, : MH // 2])
                        nc.gpsimd.tensor_tensor(
                            fold[:],
                            s[:, MH // 2 :],
                            fold[:],
                            op=mybir.AluOpType.max,
                        )
                        nc.vector.tensor_reduce(
                            nmv,
                            fold[:],
                            axis=mybir.AxisListType.X,
                            op=mybir.AluOpType.max,
                            negate=True,
                        )
                    else:
                        nc.vector.tensor_reduce(
                            nmv,
                            s[:],
                            axis=mybir.AxisListType.X,
                            op=mybir.AluOpType.max,
                            negate=True,
                        )
                    # P_h = exp(S_h - rowmax_h) -> fp16
                    pdst = p_grp[:, h, j] if bt else p_grp[:, j, h]
                    nc.scalar.activation(
                        pdst,
                        s[:],
                        mybir.ActivationFunctionType.Exp,
                        bias=cbias[:, 0:1] if cfg["noreduce"] else nmv,
                        scale=1.0,
                    )
                    if not bt and cfg["tsplit"] and not cfg["notranspose"]:
                        # ptg[p2, j, h, t, i2] = p_grp[i2, j, h, 128t+p2]
                        tout = ptg[:, h, :, j, :] if cfg["ptg2"] else ptg[:, j, h]
                        nc.sync.dma_start_transpose(out=tout, in_=p_grp[:, j, h])
                    if (bt and cfg["hout"] and not cfg["notranspose"]
                            and j == grp - 1):
                        # h-outer: this half's exps all done -> issue its
                        # transpose now, 4 exps earlier than the post-loop path
                        for jb in range(0, grp, bt):
                            nc.sync.dma_start_transpose(
                                out=ptg[:, h, jb : jb + bt],
                                in_=p_grp[:, h, jb : jb + bt],
                            )
            if cfg["notranspose"]:
                # keep a writer so ptg is allocated; PV data is garbage
                nc.vector.memset(ptg[:, 0, 0, 0, 0:1], 0.0)
            elif bt and not cfg["hout"]:
                # ptg[p2, h, j, t, i2] = p_grp[i2, h, j, 128t+p2]; one xbar
                # DMA covers bt q-tiles (src/dst contiguous in j under h-major)
                for h in range(NH):
                    teng = nc.scalar if (cfg["tring"] and h % 2) else nc.sync
                    for jb in range(0, grp, bt):
                        teng.dma_start_transpose(
                            out=ptg[:, h, jb : jb + bt], in_=p_grp[:, h, jb : jb + bt]
                        )
            elif not cfg["tsplit"]:
                assert not cfg["ptg2"], "ptg2 requires tsplit"
                nc.sync.dma_start_transpose(out=ptg[:], in_=p_grp[:])
            pend[ui] = (ptg, v_sb, ot_sb, nm_sb)

        osup = [None]  # current o super-tile and fill count

        def pv_phase(ui):
            r, p, g = units[ui]
            ptg, v_sb, ot_sb, nm_sb = pend.pop(ui)
            slots = cfg["o_slots"]
            for h in range(NH):
                # O_h^T[d, nq] = sum_t V'_t^T P^T_t  (fp16, fp32 acc);
                # V' has a ones column so row 64 is the softmax rowsum.
                # Consecutive PV outputs pack into one PSUM bank; one
                # batched PSUM->SBUF copy drains `slots` of them at once.
                if osup[0] is None:
                    osup[0] = (ps_o.tile([DR, slots, grp * 128], f32, tag="o", name=f"osup{ui}"), [])
                sup, fills = osup[0]
                sl = len(fills)
                o = sup[:, sl]
                for t in range(MT):
                    if cfg["btile"]:
                        rhs = ptg[:, h, :, t, :]
                    elif cfg["ptg2"]:
                        rhs = ptg[:, h, t]
                    else:
                        rhs = ptg[:, :, h, t, :]
                    nc.tensor.matmul(
                        o,
                        lhsT=v_sb[:, MT * h + t, :],
                        rhs=rhs,
                        start=(t == 0),
                        stop=(t == MT - 1),
                    )
                if pt_dbg is not None and r == 0 and p == 0 and h == 0 and grp == 1:
                    nc.sync.dma_start(pt_dbg[g], ptg[:, 0])
                fills.append((ot_sb, g, h))
                if len(fills) == slots:
                    # all fills of one super-tile share ot_sb and NH-major
                    # order within a group sequence; copy slot-contiguous
                    # ranges that map to contiguous (g,h) runs
                    first_ot, g0, h0 = fills[0]
                    same = all(f[0] is first_ot for f in fills)
                    if same and h0 == 0 and grp * (len(fills) // NH) * 128 > 0:
                        # fills cover q-range [grp*128*g0, ...) for both h
                        nq = grp * 128
                        qlo = nq * g0
                        dst = first_ot[
                            :, :, qlo : qlo + (slots // NH) * nq
                        ].rearrange("d h (u q) -> d u h q", q=nq)
                        src_ap = sup[:].rearrange("d (u h) q -> d u h q", h=NH)
                        if cfg["ceng"] == "vector":
                            nc.vector.tensor_copy(dst, src_ap)
                        else:
                            nc.scalar.copy(dst, src_ap)
                    else:
                        nq2 = grp * 128
                        for sl2, (otsb2, g2, h2) in enumerate(fills):
                            dst = otsb2[:, h2, nq2 * g2 : nq2 * (g2 + 1)]
                            if cfg["ceng"] == "vector":
                                nc.vector.tensor_copy(dst, sup[:, sl2])
                            else:
                                nc.scalar.copy(dst, sup[:, sl2])
                    osup[0] = None
            if g == ngrp - 1:
                ldq.dma_start(ot_d[p], ot_sb[:])
                ldq.dma_start(nm_d[p], nm_sb[:])

        depth = cfg["depth"]
        for ui in range(len(units) + depth):
            if cfg["pvfirst"] and ui >= depth:
                pv_phase(ui - depth)
            if ui < len(units):
                s_phase(ui)
            if not cfg["pvfirst"] and ui >= depth:
                pv_phase(ui - depth)

    nc.compile()
    return nc


def _get_nc(reps=1):
    key = f"nc{reps}"
    if key not in _CACHE:
        _CACHE[key] = _build_nc(reps)
    return _CACHE[key]


def _prep_inputs(q, k, v, logit_scale):
    """Host-side marshalling: scale fold, transposes, tiling, sharding."""
    scale = np.exp(
        np.minimum(logit_scale.astype(np.float32), np.float32(MAX_LOG_SCALE))
    ).reshape(H)
    qs = q.astype(np.float32) * scale[None, :, None, None].astype(np.float32)
    qT = qs.transpose(0, 1, 3, 2).reshape(B * H, D, N)
    kT = k.astype(np.float32).transpose(0, 1, 3, 2).reshape(B * H, D, N)
    qk = np.ascontiguousarray(np.stack([qT, kT], axis=2))  # [BH, D, 2, N]
    # v tiled + ones column: vt[bh, p, t, 0:64] = v[bh, 128 t + p, :], vt[...,64]=1
    vt = np.zeros((B, H, QTILES, 128, DR), np.float16)
    vt[..., D] = 1.0
    vt[..., :D] = (
        v.astype(np.float32).reshape(B, H, QTILES, 128, D).astype(np.float16)
    )
    vt = (
        np.ascontiguousarray(vt.transpose(0, 1, 3, 2, 4))
        .reshape(B * H, 128, QTILES, DR)
    )
    in_maps = []
    for c in range(NCORES):
        sl = slice(PAIRS * c, PAIRS * (c + 1))
        in_maps.append(
            {
                "qk": np.ascontiguousarray(qk[sl]),
                "vt": np.ascontiguousarray(vt[sl]),
            }
        )
    return in_maps


def _assemble(results):
    """Two-way online-softmax merge + divide + transpose back -> [B,H,N,D]."""
    out = np.empty((B * H, N, D), np.float32)
    for c in range(NCORES):
        ot = results[c]["ot"]  # [PAIRS, DR, NH, N]
        nm = results[c]["nm"]  # [PAIRS, 128, QTILES, NH]
        for p in range(PAIRS):
            bh = PAIRS * c + p
            # row r = 128*i + p2 lives at [p2, i, h]
            m_h = -nm[p].transpose(1, 0, 2).reshape(N, NH)  # per-half rowmax
            m = m_h.max(axis=1, keepdims=True)
            g = np.exp(m_h - m)  # [N, NH]
            otp = ot[p].astype(np.float32).transpose(2, 1, 0)  # [N, NH, DR]
            r_h = otp[:, :, D]  # per-half rowsum (ones column of V')
            denom = (r_h * g).sum(axis=1)  # [N]
            numer = (otp[:, :, :D] * g[:, :, None]).sum(axis=1)  # [N, D]
            out[bh] = numer / denom[:, None]
    return out.reshape(B, H, N, D)


def kernel(q, k, v, logit_scale):
    from concourse.bass_utils import run_bass_kernel_spmd

    in_maps = _prep_inputs(q, k, v, logit_scale)
    nc = _get_nc()
    res = run_bass_kernel_spmd(nc, in_maps, list(range(NCORES)))
    return _assemble(res.results)

